# revision 28
# baseline (speedup 1.0000x reference)
"""Trainium2 Bass kernel for nn_DSTA_70677981823326 (B=4, N=64, H=W=192).

Sharding (8 NeuronCores, zero cross-core communication):
  core 2s   computes output rows [0, 96)   of sample s
  core 2s+1 computes output rows [96, 192) of sample s via a vertical-flip
            parameter transform (same SPMD program, different input data).

Per-core pipeline: conv1 -> spatial/channel attention -> fuse -> mask convs
(864ch, channel-reordered) + bilinear 2x upsample -> deformable conv via a
static 3x3 tri-window (exact while |offset|<1) -> einsum -> out conv.

bf16 data path (PSUM accumulation fp32). DCN taps grouped by kernel row
(3 groups x 3 col-taps x 32ch = 96 partitions) so tap stacks load with one
4-dim affine DMA and the elementwise chain runs group-fused (FD 3*Rb*W)
in DVE 2x bf16 mode. Elementwise work split DVE / GpSimd / Scalar.
"""
import numpy as np
import ml_dtypes

import concourse.bacc as bacc
import concourse.bass as bass
import concourse.mybir as mybir
import concourse.bass_isa as bass_isa
from concourse.tile import TileContext

F32 = mybir.dt.float32
BF = mybir.dt.bfloat16
NBF = ml_dtypes.bfloat16
AF = mybir.ActivationFunctionType
ALU = mybir.AluOpType

B, N, H_FULL, W = 4, 64, 192, 192
F = 32
OM = 27 * F
WP = W + 2    # 194  x_pad width, img col c at idx c+1
AMW = W + 6   # 198  am strip width, img col c at idx c+3
XW = W + 8    # 200  x2f width, img col c at idx c+4
AM0 = 4       # strip row of image row 0
XF0 = 2       # x2f_d row of image row 0


def _geom(H):
    assert H % 4 == 0
    Hh = H // 2
    jmax = int(np.floor(Hh / 2 - 0.25)) + 1
    x3max = jmax + 1
    x2fmax = 2 * x3max + 2
    assert x2fmax + 3 <= H - 1
    return Hh, jmax, x3max, x2fmax


def _yup(r):
    j = int(np.floor(r / 2 - 0.25))
    frac = (r / 2 - 0.25) - j
    if j < 0:
        return 0, 0, 1.0, 0.0
    return j, j + 1, 1.0 - frac, frac


# ---------------------------------------------------------------------------
# host-side parameter prep
# ---------------------------------------------------------------------------

def _rk(w):
    return w[:, :, ::-1, :].copy()


def _flip_params(p):
    f = F
    q = {}
    q['conv1_w'] = _rk(p['conv1_w']); q['conv1_b'] = p['conv1_b']
    q['sa_w'] = _rk(p['sa_w'])
    q['ca_w1'] = p['ca_w1']; q['ca_w2'] = p['ca_w2']
    q['fuse_w'] = _rk(p['fuse_w']); q['fuse_b'] = p['fuse_b']
    q['down_w'] = p['down_w']; q['down_b'] = p['down_b']
    q['out_w'] = _rk(p['out_w']); q['out_b'] = p['out_b']
    q['dcn_w'] = _rk(p['dcn_w']); q['dcn_b'] = p['dcn_b']
    for nm in ('mask1', 'mask2'):
        w = p[nm + '_w']; b = p[nm + '_b']
        wn = np.empty_like(w); bn = np.empty_like(b)
        for c in range(f):
            for k in range(9):
                kp = 3 * (2 - k // 3) + k % 3
                wn[c * 18 + kp * 2 + 0] = -w[c * 18 + k * 2 + 0]
                bn[c * 18 + kp * 2 + 0] = -b[c * 18 + k * 2 + 0]
                wn[c * 18 + kp * 2 + 1] = w[c * 18 + k * 2 + 1]
                bn[c * 18 + kp * 2 + 1] = b[c * 18 + k * 2 + 1]
                wn[f * 18 + c * 9 + kp] = w[f * 18 + c * 9 + k]
                bn[f * 18 + c * 9 + kp] = b[f * 18 + c * 9 + k]
        q[nm + '_w'] = _rk(wn); q[nm + '_b'] = bn
    return q


def _om_perm():
    # channel order: typ (dy,dx,m) major, then row-group G, then col-tap t,
    # then channel c.  k = 3*G + t.
    perm = []
    for typ in range(3):
        for G in range(3):
            for t in range(3):
                k = 3 * G + t
                for c in range(F):
                    if typ == 0:
                        perm.append(c * 18 + k * 2 + 0)
                    elif typ == 1:
                        perm.append(c * 18 + k * 2 + 1)
                    else:
                        perm.append(F * 18 + c * 9 + k)
    return np.array(perm)


_PERM = _om_perm()
# 9 om blocks of 96 channels: mb = typ*3 + G
BLK_CH0 = [96 * i for i in range(9)]


def _mask_lhsT(w):
    # w [864, F, 3, 3] (channel-permuted) -> per col-tap s: [96, 864]
    out = np.zeros((3, 96, OM), np.float32)
    for s in range(3):
        for r in range(3):
            for c in range(F):
                out[s, r * 32 + c] = w[:, c, r, s]
    return out


def _prep_core(x_s, p, flipped, H):
    Hh, jmax, x3max, x2fmax = _geom(H)
    if flipped:
        x_s = x_s[:, ::-1, :].copy()
        p = _flip_params(p)
    dw4 = np.zeros((F, F, 4, 3), np.float32)
    if not flipped:
        dw4[:, :, :3] = p['down_w']
    else:
        dw4[:, :, 1:4] = p['down_w'][:, :, ::-1, :]

    d = {}
    xp = np.zeros((128, Hh + 2, WP), np.float32)
    xpad = np.zeros((N, H + 2, WP), np.float32)
    xpad[:, 1:1 + H, 1:1 + W] = x_s
    for h in range(2):
        xp[64 * h:64 * h + 64] = xpad[:, Hh * h:Hh * h + Hh + 2, :]
    d['x_pad'] = np.ascontiguousarray(xp.astype(NBF))

    colsb = {}
    piecesb = []
    colsf = {}
    piecesf = []

    def putb(name, arr, parts):
        arr = np.asarray(arr, np.float32)
        a = np.zeros((128, arr.shape[1]), np.float32)
        a[:parts] = arr
        colsb[name] = (sum(x.shape[1] for x in piecesb), arr.shape[1], parts)
        piecesb.append(a)

    def putf(name, arr, parts):
        arr = np.asarray(arr, np.float32)
        a = np.zeros((128, arr.shape[1]), np.float32)
        a[:parts] = arr
        colsf[name] = (sum(x.shape[1] for x in piecesf), arr.shape[1], parts)
        piecesf.append(a)

    c1 = np.zeros((64, 9 * 32), np.float32)
    for k in range(9):
        c1[:, k * 32:(k + 1) * 32] = p['conv1_w'][:, :, k // 3, k % 3].T
    putb('conv1', c1, 64)
    putb('ones32', np.ones((32, 1), np.float32), 32)
    saw = p['sa_w'].copy()
    saw[:, 0] /= 32.0
    sa = np.zeros((98, 32), np.float32)
    for c in range(2):
        for r in range(7):
            for s in range(7):
                sa[c * 49 + r * 7 + s] = saw[:, c, r, s]
    putb('sa', sa, 98)
    putb('fuse_a', p['fuse_w'][:, :F, 0, 0].T, 32)
    putb('fuse_b2', p['fuse_w'][:, F:, 0, 0].T, 32)
    dwl = np.zeros((96, 4 * 32), np.float32)
    for s in range(3):
        for r in range(4):
            for c in range(F):
                dwl[s * 32 + c, r * 32:(r + 1) * 32] = dw4[:, c, r, s]
    putb('down', dwl, 96)
    m1 = _mask_lhsT(p['mask1_w'][_PERM])
    m2 = _mask_lhsT(p['mask2_w'][_PERM])
    for s in range(3):
        putb(f'mask1_s{s}', m1[s], 96)
        putb(f'mask2_s{s}', m2[s], 96)
    dk = p['dcn_w'].reshape(F, F, 9)
    for G in range(3):
        arr = np.zeros((96, 32), np.float32)
        for t in range(3):
            arr[t * 32:(t + 1) * 32] = dk[:, :, 3 * G + t].T
        putb(f'dcn_g{G}', arr, 96)
    ow = np.zeros((3, 96, 64), np.float32)
    for s in range(3):
        for r in range(3):
            for c in range(F):
                ow[s, r * 32 + c] = p['out_w'][:, c, r, s]
    for s in range(3):
        putb(f'out_s{s}', ow[s], 96)

    putf('ca_w1a', (p['ca_w1'][:, :, 0, 0] / (H * W)).T, 32)
    putf('ca_w1m', p['ca_w1'][:, :, 0, 0].T, 32)
    putf('ca_w2', p['ca_w2'][:, :, 0, 0].T, 16)
    putf('conv1_b', p['conv1_b'][:, None], 32)
    putf('fuse_b', p['fuse_b'][:, None], 32)
    putf('down_b', p['down_b'][:, None], 32)
    putf('dcn_b', p['dcn_b'][:, None], 32)
    putf('out_b', p['out_b'][:, None], 64)
    btot = (p['mask1_b'] + p['mask2_b'])[_PERM]
    for i in range(9):
        putf(f'btot_{i}', btot[96 * i:96 * (i + 1)][:, None], 96)
    d['wpack_bf'] = np.ascontiguousarray(
        np.concatenate(piecesb, axis=1).astype(NBF))
    d['wpack_f32'] = np.ascontiguousarray(np.concatenate(piecesf, axis=1))
    return d, (colsb, colsf)


# ---------------------------------------------------------------------------
# kernel emission
# ---------------------------------------------------------------------------

def emit(H, wcols, wtots):
    (colsb, colsf) = wcols
    (wtot_bf, wtot_f) = wtots
    Hh, jmax, x3max, x2fmax = _geom(H)
    nc = bacc.Bacc(None, target_bir_lowering=False)

    x_pad_d = nc.dram_tensor("x_pad", [128, Hh + 2, WP], BF, kind="ExternalInput")
    wbf_d = nc.dram_tensor("wpack_bf", [128, wtot_bf], BF, kind="ExternalInput")
    wf_d = nc.dram_tensor("wpack_f32", [128, wtot_f], F32, kind="ExternalInput")
    out_d = nc.dram_tensor("out", [64, Hh, W], F32, kind="ExternalOutput")
    x2_d = nc.dram_tensor("x2_scr", [32, H, W], BF)
    am_rows = x2fmax + 8                      # strip rows: image rows -4..x2fmax+3
    am_d = nc.dram_tensor("am_scr", [2, am_rows * AMW], BF)
    XROWS = x2fmax + 3                        # x2f rows: image rows -2..x2fmax
    # x2f replicas with tap shift baked in, so phase-C stack loads are single
    # 3-dim DMAs: xt1 block t = x2f cols shifted by t; xt2 block s = rows
    # shifted by s.
    xt1_d = nc.dram_tensor("xt1_scr", [96, XROWS, XW], BF)
    xt2_d = nc.dram_tensor("xt2_scr", [96, XROWS, XW], BF)

    def wsl(wt, cols, name, parts=None, c0=0, cn=None):
        o, n, pts = cols[name]
        if parts is None:
            parts = pts
        if cn is None:
            cn = n - c0
        return wt[0:parts, o + c0:o + c0 + cn]

    strip_max = x2fmax + 3                    # last image row needed in strip

    with TileContext(nc) as tc:
        with (
            tc.tile_pool(name="wt", bufs=1) as wpool,
            tc.tile_pool(name="const", bufs=1) as cpool,
        ):
            wtb = wpool.tile([128, wtot_bf], BF)
            wtf = wpool.tile([128, wtot_f], F32)
            nc.gpsimd.dma_start(out=wtb[:], in_=wbf_d[:])
            nc.gpsimd.dma_start(out=wtf[:], in_=wf_d[:])

            def Wb(name, **kw):
                return wsl(wtb, colsb, name, **kw)

            def Wf(name, **kw):
                return wsl(wtf, colsf, name, **kw)

            # zero pads: am strip rows img -4..-1 ; x2f replica pad rows
            ztile = cpool.tile([96, 4 * AMW], BF)
            nc.gpsimd.memset(ztile[:], 0.0)
            nc.sync.dma_start(
                out=bass.AP(am_d, 0, [[am_rows * AMW, 2], [1, 4 * AMW]]),
                in_=ztile[0:2, :])
            nc.sync.dma_start(out=xt1_d[:, 0:2, :], in_=ztile[0:96, 0:2 * XW])
            nc.sync.dma_start(out=xt2_d[0:32, 0:2, :], in_=ztile[0:32, 0:2 * XW])
            nc.sync.dma_start(out=xt2_d[32:64, 0:1, :], in_=ztile[0:32, 0:XW])

            # ------------- Phase A: conv1 + pools -------------
            nbA = H // 2
            mxbuf = cpool.tile([32, nbA], F32)
            smbuf = cpool.tile([32, nbA], F32)
            gate = cpool.tile([32, 1], F32)
            with (
                tc.tile_pool(name="pA", bufs=2) as pool,
                tc.tile_pool(name="pX", bufs=2) as xpool_a,
                tc.tile_pool(name="pSt", bufs=2) as stpool,
                tc.tile_pool(name="psA", bufs=2, space="PSUM") as psum,
                tc.tile_pool(name="psS", bufs=2, space="PSUM") as psumS,
                tc.tile_pool(name="psG", bufs=1, space="PSUM") as psumG,
            ):
                Hq = Hh // 2
                for q in range(4):
                    h = q // 2
                    r0 = Hq * (q % 2)
                    xsb = xpool_a.tile([64, Hq + 2, WP], BF, tag="xsb")
                    nc.sync.dma_start(out=xsb[:],
                                      in_=x_pad_d[64 * h:64 * h + 64,
                                                  r0:r0 + Hq + 2, :])
                    for bg in range(Hq // 8):      # stage groups of 4 blocks
                        y0g = Hh * h + r0 + 8 * bg
                        do_strip = y0g <= strip_max
                        x2st = pool.tile([32, 8, W], BF, tag="x2st")
                        if do_strip:
                            stav = stpool.tile([1, 8, AMW], BF, tag="stav")
                            stmx = stpool.tile([32, 8, AMW], BF, tag="stmx")
                            nc.gpsimd.memset(stav[:, :, 0:3], 0.0)
                            nc.gpsimd.memset(stav[:, :, 195:198], 0.0)
                            nc.gpsimd.memset(stmx[0:1, :, 0:3], 0.0)
                            nc.gpsimd.memset(stmx[0:1, :, 195:198], 0.0)
                        for bq in range(4):
                            y0 = y0g + 2 * bq
                            band = y0 // 2
                            yl = 8 * bg + 2 * bq
                            ps = psum.tile([32, 2, W], F32, tag="psc1")
                            for k in range(9):
                                r, s = k // 3, k % 3
                                rhs = xsb[:, yl + r:yl + r + 2, s:s + W]
                                nc.tensor.matmul(ps[:], Wb('conv1', c0=k * 32, cn=32),
                                                 rhs, start=(k == 0), stop=(k == 8))
                            x2b = x2st[:, 2 * bq:2 * bq + 2, :]
                            nc.scalar.activation(x2b, ps[:], AF.Relu,
                                                 bias=Wf('conv1_b'),
                                                 accum_out=smbuf[:, band:band + 1])
                            nc.vector.tensor_reduce(mxbuf[:, band:band + 1], x2b,
                                                    axis=mybir.AxisListType.XY,
                                                    op=ALU.max)
                            if do_strip and y0 <= strip_max:
                                nc.gpsimd.partition_all_reduce(
                                    stmx[:, 2 * bq:2 * bq + 2, 3:3 + W], x2b,
                                    channels=32,
                                    reduce_op=bass_isa.ReduceOp.max)
                        if do_strip:
                            for bq in range(4):
                                pss = psumS.tile([1, 2, W], F32, tag="pss")
                                nc.tensor.matmul(pss[:], Wb('ones32'),
                                                 x2st[:, 2 * bq:2 * bq + 2, :],
                                                 start=True, stop=True)
                                nc.scalar.activation(
                                    stav[0:1, 2 * bq:2 * bq + 2, 3:3 + W],
                                    pss[:], AF.Copy)
                        if y0g <= 102:
                            nc.sync.dma_start(out=x2_d[:, y0g:y0g + 8, :],
                                              in_=x2st[:])
                        if do_strip:
                            nc.sync.dma_start(
                                out=bass.AP(am_d, (AM0 + y0g) * AMW,
                                            [[1, 8 * AMW]]),
                                in_=stav[0:1, :, :])
                            nc.sync.dma_start(
                                out=bass.AP(am_d,
                                            am_rows * AMW + (AM0 + y0g) * AMW,
                                            [[1, 8 * AMW]]),
                                in_=stmx[0:1, :, :])
                # channel-attention gate
                apv = cpool.tile([32, 1], F32)
                mpv = cpool.tile([32, 1], F32)
                with nc.allow_low_precision(reason="f32 accum"):
                    nc.vector.tensor_reduce(apv[:], smbuf[:],
                                            axis=mybir.AxisListType.X, op=ALU.add)
                nc.vector.tensor_reduce(mpv[:], mxbuf[:], axis=mybir.AxisListType.X,
                                        op=ALU.max)
                psg = psumG.tile([32, 1], F32, tag="psg")
                hts = []
                for nm, vec in (('ca_w1a', apv), ('ca_w1m', mpv)):
                    ph = psumG.tile([16, 1], F32, tag="ph" + nm)
                    nc.tensor.matmul(ph[:], Wf(nm), vec[:], start=True, stop=True)
                    ht = cpool.tile([16, 1], F32, tag="ht" + nm)
                    nc.scalar.activation(ht[:], ph[:], AF.Relu)
                    hts.append(ht)
                for i, ht in enumerate(hts):
                    nc.tensor.matmul(psg[:], Wf('ca_w2'), ht[:],
                                     start=(i == 0), stop=(i == 1))
                nc.scalar.activation(gate[:], psg[:], AF.Sigmoid)

            # ------------- Phase B: sa silu (to SBUF) then fuse -> x2f ----
            with (
                tc.tile_pool(name="pB", bufs=2) as pool,
                tc.tile_pool(name="pXS", bufs=1) as xspool,
                tc.tile_pool(name="psB", bufs=2, space="PSUM") as psum,
            ):
                RB = 16
                x2s_sb = xspool.tile([32, x2fmax + 4, W], BF)
                bands_b = []
                yb = 0
                while yb <= x2fmax:
                    bands_b.append((yb, min(RB, x2fmax + 1 - yb)))
                    yb += RB
                for yb, rows in bands_b:
                    t98 = pool.tile([98, RB, W], BF, tag="t98")
                    for c in range(2):
                        for r in range(7):
                            src = bass.AP(
                                am_d,
                                c * am_rows * AMW + (AM0 + yb - 3 + r) * AMW,
                                [[1, 7], [AMW, rows], [1, W]])
                            nc.sync.dma_start(
                                out=t98[c * 49 + r * 7:c * 49 + r * 7 + 7,
                                        0:rows, :],
                                in_=src)
                    for h0 in range(0, rows, 2):
                        hn = min(2, rows - h0)
                        ps = psum.tile([32, 2, W], F32, tag="pssa")
                        nc.tensor.matmul(ps[:, 0:hn, :], Wb('sa'),
                                         t98[:, h0:h0 + hn, :], start=True,
                                         stop=True)
                        nc.scalar.activation(
                            x2s_sb[:, yb + h0:yb + h0 + hn, :],
                            ps[:, 0:hn, :], AF.Silu)
                for yb, rows in bands_b:
                    x2r = pool.tile([32, RB, W], BF, tag="x2r")
                    nc.sync.dma_start(out=x2r[:, 0:rows, :],
                                      in_=x2_d[:, yb:yb + rows, :])
                    x2c = pool.tile([32, RB, W], BF, tag="x2c")
                    nc.scalar.activation(x2c[:, 0:rows, :], x2r[:, 0:rows, :],
                                         AF.Copy, scale=gate[:])
                    x2ft = pool.tile([32, RB, XW], BF, tag="x2ft")
                    nc.gpsimd.memset(x2ft[:, :, 0:4], 0.0)
                    nc.gpsimd.memset(x2ft[:, :, 196:200], 0.0)
                    for h0 in range(0, rows, 2):
                        hn = min(2, rows - h0)
                        ps2 = psum.tile([32, 2, W], F32, tag="psfu")
                        nc.tensor.matmul(ps2[:, 0:hn, :], Wb('fuse_a'),
                                         x2s_sb[:, yb + h0:yb + h0 + hn, :],
                                         start=True, stop=False)
                        nc.tensor.matmul(ps2[:, 0:hn, :], Wb('fuse_b2'),
                                         x2c[:, h0:h0 + hn, :],
                                         start=False, stop=True)
                        nc.scalar.activation(x2ft[:, h0:h0 + hn, 4:4 + W],
                                             ps2[:, 0:hn, :], AF.Relu,
                                             bias=Wf('fuse_b'))
                    r0 = XF0 + yb
                    for t in range(3):
                        nc.sync.dma_start(
                            out=xt1_d[t * 32:(t + 1) * 32, r0:r0 + rows,
                                      0:XW - t],
                            in_=x2ft[:, 0:rows, t:XW])
                    for s in range(3):
                        nc.sync.dma_start(
                            out=xt2_d[s * 32:(s + 1) * 32,
                                      r0 - s:r0 - s + rows, :],
                            in_=x2ft[:, 0:rows, :])

            # ------------- Phase C: DCN bands (Rb=2) -------------
            R = 4
            bands = []
            rb = 0
            while rb <= Hh:
                bands.append((rb, min(rb + R, Hh + 1)))
                rb = bands[-1][1]

            import contextlib
            with contextlib.ExitStack() as _st:
                pool = _st.enter_context(tc.tile_pool(name="pC", bufs=2))
                wpoolc = _st.enter_context(tc.tile_pool(name="pW", bufs=1))
                prpool = _st.enter_context(tc.tile_pool(name="pPr", bufs=2))
                ompool = _st.enter_context(tc.tile_pool(name="pOm", bufs=2))
                scpool = _st.enter_context(tc.tile_pool(name="pS", bufs=1))
                dfpool = _st.enter_context(tc.tile_pool(name="pD", bufs=1))
                d3pool = _st.enter_context(tc.tile_pool(name="pD3", bufs=1))
                o2pool = _st.enter_context(tc.tile_pool(name="pO2", bufs=1))
                aspool = _st.enter_context(tc.tile_pool(name="pAs", bufs=2))
                dspool = _st.enter_context(tc.tile_pool(name="pDs", bufs=2))
                pool3 = _st.enter_context(tc.tile_pool(name="pC3", bufs=2))
                xpool = _st.enter_context(tc.tile_pool(name="xup", bufs=3))
                x3pool = _st.enter_context(tc.tile_pool(name="x3p", bufs=1))
                omqpool = _st.enter_context(tc.tile_pool(name="omq", bufs=2))
                psum = _st.enter_context(tc.tile_pool(name="psC", bufs=1, space="PSUM"))
                psumM = _st.enter_context(tc.tile_pool(name="psM", bufs=2, space="PSUM"))
                psumE = _st.enter_context(tc.tile_pool(name="psE", bufs=2, space="PSUM"))
                psumO = _st.enter_context(tc.tile_pool(name="psO", bufs=2, space="PSUM"))
                x3_pad = x3pool.tile([32, x3max + 2, 100], BF)
                nc.gpsimd.memset(x3_pad[:], 0.0)
                x3_done = [-1]
                omq_done = {}
                xup_cache = {}
                dcn_prev = [None]

                def ensure_x3(rmax):
                    while x3_done[0] < min(rmax, x3max):
                        q0 = x3_done[0] + 1
                        rows = min(4, x3max + 1 - q0)
                        wr0 = 2 * q0 - 1
                        wrn = 2 * rows + 2
                        r96d = d3pool.tile([96, 10, XW], BF, tag="r96d")
                        nc.sync.dma_start(
                            out=r96d[:, 0:wrn, :],
                            in_=xt1_d[:, XF0 + wr0:XF0 + wr0 + wrn, :])
                        ps = psum.tile([32, 4, 96], F32, tag="psx3")
                        for r in range(4):
                            rhs = r96d[0:96, r:r + 2 * (rows - 1) + 1:2,
                                       3:3 + 2 * 95 + 1:2]
                            nc.tensor.matmul(ps[:, 0:rows, :],
                                             Wb('down', c0=r * 32, cn=32), rhs,
                                             start=(r == 0), stop=(r == 3))
                        nc.scalar.activation(
                            x3_pad[:, 1 + q0:1 + q0 + rows, 2:98],
                            ps[:, 0:rows, :], AF.Relu, bias=Wf('down_b'))
                        x3_done[0] = q0 + rows - 1

                def ensure_omq(p_):
                    if p_ in omq_done:
                        return omq_done[p_]
                    rows = min(4, jmax + 1 - 4 * p_)
                    ensure_x3(4 * p_ + rows)
                    qt = omqpool.tile([96, 3, 3, 4, 96], BF, tag="omq")
                    r96o = pool3.tile([96, 4, 100], BF, tag="r96o")
                    for s in range(3):
                        nc.sync.dma_start(
                            out=r96o[s * 32:(s + 1) * 32, 0:rows, :],
                            in_=x3_pad[:, 4 * p_ + s:4 * p_ + s + rows, :])
                    for mb in range(9):
                        typ, G = mb // 3, mb % 3
                        ps = psum.tile([96, 4, 96], F32, tag="psomq")
                        for s in range(3):
                            rhs = r96o[0:96, 0:rows, 1 + s:97 + s]
                            nc.tensor.matmul(
                                ps[:, 0:rows, :],
                                Wb(f'mask2_s{s}', parts=96, c0=BLK_CH0[mb], cn=96),
                                rhs, start=(s == 0), stop=(s == 2))
                        nc.scalar.activation(qt[:, typ, G, 0:rows, :],
                                             ps[:, 0:rows, :], AF.Copy)
                    omq_done[p_] = qt
                    if p_ - 2 in omq_done:
                        del omq_done[p_ - 2]
                    return qt

                def xup_row(j):
                    # returns (0.25*xt_j, 0.75*xt_j): the y-interp weights are
                    # always {0.25, 0.75} (or {1, 0} at y=0, where j1==j2 and
                    # xt25+xt75 == xt).
                    if j in xup_cache:
                        return xup_cache[j]
                    qt = ensure_omq(j // 4)
                    rr = j - 4 * (j // 4)
                    xt75 = xpool.tile([96, 3, 3, W], BF, tag="xup75")
                    xt25 = xpool.tile([96, 3, 3, W], BF, tag="xup25")
                    q75 = scpool.tile([96, 3, 3, 96], BF, tag="q75")
                    q25 = scpool.tile([96, 3, 3, 96], BF, tag="q25")
                    nc.scalar.activation(q75[:], qt[:, :, :, rr, :], AF.Copy,
                                         scale=0.75)
                    nc.scalar.activation(q25[:], qt[:, :, :, rr, :], AF.Copy,
                                         scale=0.25)
                    nc.vector.tensor_tensor(xt75[:, :, :, 2::2],
                                            q75[:, :, :, 1:96],
                                            q25[:, :, :, 0:95], op=ALU.add)
                    nc.vector.tensor_tensor(xt75[:, :, :, 1:191:2],
                                            q75[:, :, :, 0:95],
                                            q25[:, :, :, 1:96], op=ALU.add)
                    nc.scalar.activation(xt75[:, :, :, 0:1],
                                         qt[:, :, :, rr, 0:1], AF.Copy)
                    nc.scalar.activation(xt75[:, :, :, 191:192],
                                         qt[:, :, :, rr, 95:96], AF.Copy)
                    nc.vector.tensor_scalar_mul(xt25[:], xt75[:], 0.25)
                    nc.vector.tensor_scalar_mul(xt75[:], xt75[:], 0.75)
                    xup_cache[j] = (xt25, xt75)
                    return xup_cache[j]

                for bi, (rb, re) in enumerate(bands):
                    Rb = re - rb
                    need = sorted({j for y in range(rb, re) for j in _yup(y)[:2]})
                    need = [j for j in need if j <= jmax]
                    for j in need:
                        xup_row(j)
                    for j in list(xup_cache):
                        if j < need[0]:
                            del xup_cache[j]
                    # prep: single 8-row tile (tap-col shift baked into xt1
                    # partitions); the chain reads (G, i) row lattices via
                    # overlapping-row APs.  prepL is shifted one col left so
                    # both horizontal diffs read even-aligned (DVE 2x mode).
                    prep = prpool.tile([96, R + 4, 196], BF, tag="prep")
                    prepL = prpool.tile([96, R + 4, 196], BF, tag="prepL")
                    r0 = XF0 + rb - 2
                    nc.sync.dma_start(out=prep[:, 0:Rb + 4, :],
                                      in_=xt1_d[:, r0:r0 + Rb + 4, 1:197])
                    nc.sync.dma_start(out=prepL[:, 0:Rb + 4, :],
                                      in_=xt1_d[:, r0:r0 + Rb + 4, 0:196])

                    def ovl(tile, ss, col0, rw, wdt):
                        fa = tile[:]
                        return bass.AP(fa.tensor, fa.offset + ss * rw + col0,
                                       [fa.ap[0], [rw, 3], [rw, Rb], [1, wdt]])
                    r96m = pool3.tile([96, R, XW], BF, tag="r96m")
                    nc.sync.dma_start(
                        out=r96m[:, 0:Rb, :],
                        in_=xt2_d[:, XF0 + rb - 1:XF0 + rb - 1 + Rb, :])
                    # om2 upsample rows: a_ is always 0.25 (even y) or 0.75
                    # (odd y) or 1.0 (y=0, j1==j2 so xt25+xt75 works there too)
                    om2u = o2pool.tile([96, 3, 3, R, W], BF, tag="om2u")
                    for i, y in enumerate(range(rb, re)):
                        j1, j2, a_, b_ = _yup(y)
                        j2 = min(j2, jmax)
                        assert (a_, b_) in ((0.25, 0.75), (0.75, 0.25), (1.0, 0.0))
                        xa = xup_row(j1)[0 if a_ == 0.25 else 1]
                        xb = xup_row(j2)[1 if a_ == 0.25 else 0]
                        nc.vector.tensor_tensor(om2u[:, :, :, i, :], xa[:],
                                                xb[:], op=ALU.add)
                    # mask1 conv + bias drains, then in-place add of om2u
                    om = ompool.tile([96, 3, 3, R, W], BF, tag="om")
                    for mb in range(9):
                        typ, G = mb // 3, mb % 3
                        for i0 in range(0, Rb, 2):
                            hn = min(2, Rb - i0)
                            ps = psumM.tile([96, 2, W], F32, tag="psom1")
                            for s in range(3):
                                rhs = r96m[0:96, i0:i0 + hn, 3 + s:3 + s + W]
                                nc.tensor.matmul(
                                    ps[:, 0:hn, :],
                                    Wb(f'mask1_s{s}', parts=96,
                                       c0=BLK_CH0[mb], cn=96),
                                    rhs, start=(s == 0), stop=(s == 2))
                            nc.scalar.activation(om[:, typ, G, i0:i0 + hn, :],
                                                 ps[:, 0:hn, :], AF.Identity,
                                                 bias=Wf(f'btot_{mb}', parts=96))
                    nc.vector.tensor_tensor(om[:, :, :, 0:Rb, :],
                                            om[:, :, :, 0:Rb, :],
                                            om2u[:, :, :, 0:Rb, :], op=ALU.add)
                    # weights + mask gate
                    # wyn = min(dy,0) = -relu(-dy); the sign is folded into
                    # the combine subtracts below.
                    wyp = wpoolc.tile([96, 3, R, W], BF, tag="wyp")
                    wyn = wpoolc.tile([96, 3, R, W], BF, tag="wyn")
                    wxp = wpoolc.tile([96, 3, R, W], BF, tag="wxp")
                    wxn = wpoolc.tile([96, 3, R, W], BF, tag="wxn")
                    sg = wpoolc.tile([96, 3, R, W], BF, tag="sg")
                    ody = om[:, 0, :, 0:Rb, :]
                    odx = om[:, 1, :, 0:Rb, :]
                    nc.vector.tensor_scalar_max(wyp[:, :, 0:Rb, :], ody, 0.0)
                    nc.vector.tensor_scalar_min(wyn[:, :, 0:Rb, :], ody, 0.0)
                    nc.vector.tensor_scalar_max(wxp[:, :, 0:Rb, :], odx, 0.0)
                    nc.vector.tensor_scalar_min(wxn[:, :, 0:Rb, :], odx, 0.0)
                    nc.scalar.activation(sg[:, :, 0:Rb, :], om[:, 2, :, 0:Rb, :],
                                         AF.Sigmoid)
                    # horizontal diffs (DVE 2x: all operands even-aligned)
                    dxm = dfpool.tile([96, R + 4, W], BF, tag="dxm")
                    dxp = dfpool.tile([96, R + 4, W], BF, tag="dxp")
                    nc.vector.tensor_tensor(dxm[:, 0:Rb + 4, :],
                                            prepL[:, 0:Rb + 4, 2:194],
                                            prep[:, 0:Rb + 4, 2:194],
                                            op=ALU.subtract)
                    nc.vector.tensor_tensor(dxp[:, 0:Rb + 4, :],
                                            prepL[:, 0:Rb + 4, 4:196],
                                            prep[:, 0:Rb + 4, 2:194],
                                            op=ALU.subtract)
                    # group-fused tri-window chain
                    As = aspool.tile([96, 3, 3, R, W], BF, tag="As")
                    t1 = scpool.tile([96, 3, R, W], BF, tag="t1")
                    for ss in range(3):
                        a_t = As[:, ss, :, 0:Rb, :]
                        nc.vector.tensor_tensor(t1[:, :, 0:Rb, :],
                                                wxn[:, :, 0:Rb, :],
                                                ovl(dxm, ss, 0, W, W),
                                                op=ALU.mult)
                        nc.vector.tensor_tensor(a_t, wxp[:, :, 0:Rb, :],
                                                ovl(dxp, ss, 0, W, W),
                                                op=ALU.mult)
                        nc.vector.tensor_tensor(a_t, a_t, t1[:, :, 0:Rb, :],
                                                op=ALU.subtract)
                        nc.vector.tensor_tensor(a_t, a_t,
                                                ovl(prep, ss, 2, 196, W),
                                                op=ALU.add)
                    A0 = As[:, 0, :, 0:Rb, :]
                    A1 = As[:, 1, :, 0:Rb, :]
                    A2 = As[:, 2, :, 0:Rb, :]
                    nc.vector.tensor_tensor(A0, A0, A1, op=ALU.subtract)
                    nc.vector.tensor_tensor(A2, A2, A1, op=ALU.subtract)
                    nc.vector.tensor_tensor(A0, A0, wyn[:, :, 0:Rb, :],
                                            op=ALU.mult)
                    nc.vector.tensor_tensor(A2, A2, wyp[:, :, 0:Rb, :],
                                            op=ALU.mult)
                    nc.vector.tensor_tensor(A1, A1, A0, op=ALU.subtract)
                    nc.vector.tensor_tensor(A1, A1, A2, op=ALU.add)
                    nc.vector.tensor_tensor(A1, A1, sg[:, :, 0:Rb, :],
                                            op=ALU.mult)
                    # einsum accumulate + dcn slot
                    dslot = dspool.tile([32, R + 2, WP], BF, tag="dslot")
                    nc.gpsimd.memset(dslot[:, :, 0:1], 0.0)
                    nc.gpsimd.memset(dslot[:, :, 193:194], 0.0)
                    if bi == 0:
                        nc.gpsimd.memset(dslot[:, 0:2, :], 0.0)
                    if bi > 0:
                        pR = bands[bi - 1][1] - bands[bi - 1][0]
                        nc.vector.tensor_copy(dslot[:, 0:2, :],
                                              dcn_prev[0][:, pR:pR + 2, :])
                    for i0 in range(0, Rb, 2):
                        hn = min(2, Rb - i0)
                        pse = psumE.tile([32, 2, W], F32, tag="pse")
                        for G in range(3):
                            nc.tensor.matmul(pse[:, 0:hn, :], Wb(f'dcn_g{G}'),
                                             As[:, 1, G, i0:i0 + hn, :],
                                             start=(G == 0), stop=(G == 2))
                        nc.scalar.activation(dslot[:, 2 + i0:2 + i0 + hn, 1:1 + W],
                                             pse[:, 0:hn, :], AF.Relu,
                                             bias=Wf('dcn_b'))
                    dcn_prev[0] = dslot
                    ob0 = max(rb - 1, 0)
                    orows = (re - 1) - ob0
                    if bi == len(bands) - 1:
                        orows = Hh - ob0
                    if orows <= 0:
                        continue
                    so = ob0 - (rb - 2)
                    r96t = pool3.tile([96, R, WP], BF, tag="r96t")
                    for r in range(3):
                        nc.sync.dma_start(
                            out=r96t[r * 32:(r + 1) * 32, 0:orows, :],
                            in_=dslot[:, so - 1 + r:so - 1 + r + orows, :])
                    outt = dspool.tile([64, R, W], F32, tag="outt")
                    for i0 in range(0, orows, 2):
                        hn = min(2, orows - i0)
                        pso = psumO.tile([64, 2, W], F32, tag="psout")
                        for s in range(3):
                            rhs = r96t[0:96, i0:i0 + hn, s:s + W]
                            nc.tensor.matmul(pso[:, 0:hn, :], Wb(f'out_s{s}'),
                                             rhs, start=(s == 0), stop=(s == 2))
                        nc.scalar.activation(outt[:, i0:i0 + hn, :],
                                             pso[:, 0:hn, :], AF.Relu,
                                             bias=Wf('out_b'))
                    nc.sync.dma_start(out=out_d[:, ob0:ob0 + orows, :],
                                      in_=outt[:, 0:orows, :])

    nc.finalize()
    return nc


# ---------------------------------------------------------------------------
# public entry
# ---------------------------------------------------------------------------

_CACHE = {}


def _compiled(H, wcols, wtots):
    key = H
    if key not in _CACHE:
        _CACHE[key] = emit(H, wcols, wtots)
    return _CACHE[key]


def kernel(**inputs):
    from concourse.bass_utils import run_bass_kernel_spmd
    H = H_FULL
    Hh = H // 2
    x = np.asarray(inputs['x'], np.float32)
    p = {k: np.asarray(v, np.float32) for k, v in inputs.items() if k != 'x'}
    in_maps = []
    wcols = None
    wtots = None
    for core in range(8):
        d, cols = _prep_core(x[core // 2], p, core % 2 == 1, H)
        wcols = cols
        wtots = (d['wpack_bf'].shape[1], d['wpack_f32'].shape[1])
        in_maps.append(d)
    nc = _compiled(H, wcols, wtots)
    res = run_bass_kernel_spmd(nc, in_maps, list(range(8))).results
    out = np.zeros((B, N, H, W), np.float32)
    for core in range(8):
        o = res[core]['out'].reshape(N, Hh, W)
        if core % 2:
            out[core // 2, :, Hh:] = o[:, ::-1, :]
        else:
            out[core // 2, :, :Hh] = o
    return out
